# revision 5
# baseline (speedup 1.0000x reference)
"""Trainium2 Bass kernel for the Dombi t-norm feature-expansion module.

Computation (per reference):
    t = (1/x - 1) ** lam                       # [B, 16]
    s = t @ M.T                                # subset sums, M = binary mask [2500, 16]
    h = 1 / (1 + s ** (1/lam))                 # [B, 2500]
    out = concat([x, h], axis=1)               # [B, 2516]

Strategy (8 NeuronCores, pure data parallel over batch; fp16 output,
upcast to fp32 on the host — well inside the 2e-2 rel-err budget):
  - per core shard of 4096 rows: 8 groups x (4 tiles of 128 rows)
  - t = exp(lam * ln(1/x-1)): DVE reciprocal_approx_fast + ACT Ln + ACT
    Exp (Ln and Exp share the natural_log_exp table set -> no extra
    table loads); hi/lo bf16 split + PE transpose for fp32-grade dots
  - PE: K=32 matmuls fold hi+lo, 4 strips packed via tile_position
  - h: ACT pass 1 computes w = Ln(scale*s) for ALL columns (scale
    centers w around 0 to halve its fp16 quantization error); then the
    columns of each tile are split between two engines:
      * cols [DC:2500] -> ACT Sigmoid(-1/lam * (w + CTR))  (one pass)
      * cols [0:DC]    -> DVE: u = s^(1/lam) via Schraudolph exp2 with
        mantissa correction (int16 bit tricks + fp16 quadratic), then
        h = 1/(1+u) via bit-trick seed + one Newton step.  9 stock DVE
        ops, 5 of them at 4x rate (fp16/int16), 4 at 2x.
    DC balances the two engines (ACT ~2/5 sigmoid + full ln vs DVE).
  - ln/sigmoid live in different ACT table sets -> groups are phased
    [4,4] so only 4 table switches (~10.6us) per iteration
  - output streams out per 128-row tile as fp16 (20.6 MB/core)
Predicted: ACT ~130us, DVE ~122us, DMA ~60us, PE ~36us -> ~135us/core.
"""

import os
import sys
from itertools import combinations

import numpy as np

_REPO_CANDIDATES = ("/opt/trn_rl_repo", "/root/.axon_site/_ro/trn_rl_repo")


def _ensure_concourse():
    try:
        import concourse.bass  # noqa: F401
        return
    except ImportError:
        pass
    for p in _REPO_CANDIDATES:
        if os.path.isdir(p) and p not in sys.path:
            sys.path.insert(0, p)
    import concourse.bass  # noqa: F401


B, N, ADD = 32768, 16, 4
NCORES = 8
BC = B // NCORES            # 4096 rows per core
S = 2500                    # number of subsets (sizes 2..4 of 16)
SP = S                      # per-tile column stride in the w tile
OUTC = N + S                # 2516 output columns
TILES_PER_GROUP = 4         # batch tiles of 128 per PE pass
GROUPS = BC // (128 * TILES_PER_GROUP)   # 8
CHUNKS = (512, 512, 512, 512, 452)       # moving-operand chunk widths
DC = int(os.environ.get("DOMBI_DC", "1056"))  # DVE-path columns per tile
PHASES = tuple(
    int(t) for t in os.environ.get("DOMBI_PHASES", "4,4").split(",")
)
W_BUFS = int(os.environ.get("DOMBI_WBUFS", "6"))

# exp2 mantissa-correction quadratic: alpha*(m^2 + beta*m + gamma)
# ~= 2^(m-1)/m on [1,2), rel minimax ~3.5e-3 (numpy fit)
EXP2_ALPHA = 0.23368115
EXP2_BETA = -2.97030264
EXP2_GAMMA = 6.23478840
# fp16 reciprocal seed: r0 = bitcast(K2 - bits(d)), ~5.1% over d in
# [1.4, 2050]; one Newton with slight over-relaxation recovers ~1.4e-3
RECIP_K2 = 0x7798
NEWTON_TWO = 2.0013


def _build_mask_rep():
    """[128, 2500] bf16: M.T tiled 8x vertically (4 strips x {hi, lo})."""
    import ml_dtypes
    rows = []
    for i in range(2, ADD + 1):
        for c in combinations(range(N), i):
            r = np.zeros(N, dtype=np.float32)
            r[list(c)] = 1.0
            rows.append(r)
    M = np.stack(rows)                       # [2500, 16]
    MT = M.T.astype(np.float32)              # [16, 2500]
    rep = np.zeros((128, S), dtype=np.float32)
    for j in range(TILES_PER_GROUP):
        rep[32 * j: 32 * j + 16] = MT        # hi half of the strip
        rep[32 * j + 16: 32 * j + 32] = MT   # lo half of the strip
    return rep.astype(ml_dtypes.bfloat16)


def _emit_kernel(tc, x, mask, out, lam, reps=0, dummy=None):
    import concourse.bass as bass  # noqa: F401
    from concourse import mybir
    from concourse.masks import make_identity
    from contextlib import ExitStack

    if reps:
        # benchmark mode: run the whole body `reps` times in a HW loop
        unroll = 4 if reps % 4 == 0 else 1
        with tc.For_i(0, reps // unroll, 1):
            for _ in range(unroll):
                _emit_kernel(tc, x, mask, out, lam, reps=0, dummy=dummy)
        return

    nc = tc.nc
    f32 = mybir.dt.float32
    f16 = mybir.dt.float16
    i16 = mybir.dt.int16
    bf16 = mybir.dt.bfloat16
    AF = mybir.ActivationFunctionType
    inv_lam = 1.0 / lam                      # 10/3 for lam=0.3
    c2 = inv_lam / float(np.log(2.0))        # base-2 exponent multiplier

    # center of the ln(s) range (s = sums of 2..4 terms of (1/x-1)^lam,
    # x in (0.05, 0.95)) -- the Ln pass computes w' = ln(s) - CTR so the
    # fp16 w' sits near 0 (half the quantization error of raw ln s)
    vmin, vmax = 1.0 / 0.95 - 1.0, 1.0 / 0.05 - 1.0
    wlo, whi = vmin ** lam, vmax ** lam
    ln_lo, ln_hi = float(np.log(2 * wlo)), float(np.log(ADD * whi))
    CTR = 0.5 * (ln_lo + ln_hi)
    LN_SCALE = float(np.exp(-CTR))

    # Schraudolph step: i16 = round(S1*w' + S2) gives fp16 bits of
    # ~2^(c2*(w'+CTR)) = s^(1/lam)
    SCH_S1 = 1024.0 * c2
    SCH_S2 = 1024.0 * (15.0 + c2 * CTR)

    ktiles = BC // 128                      # 32 batch tiles of 128 rows
    with ExitStack() as ctx:
        singles = ctx.enter_context(tc.tile_pool(name="singles", bufs=1))
        stagep = ctx.enter_context(tc.tile_pool(name="stagep", bufs=2))
        wp = ctx.enter_context(tc.tile_pool(name="wp", bufs=GROUPS))
        up = ctx.enter_context(tc.tile_pool(name="up", bufs=W_BUFS))
        dvp = ctx.enter_context(tc.tile_pool(name="dvp", bufs=2))
        psum = ctx.enter_context(tc.tile_pool(name="psum", bufs=2, space="PSUM"))

        # whole x shard resident in SBUF: x_big[p, 16k+n] = x[128k+p, n]
        x_big = singles.tile([128, ktiles * N], f32, name="x_big")
        xb_r = x_big.rearrange("p (k n) -> p k n", n=N)
        x_src = x.rearrange("(k p) n -> p k n", p=128)
        kq = ktiles // 4
        for q in range(4):
            nc.sync.dma_start(
                out=xb_r[:, q * kq:(q + 1) * kq, :],
                in_=x_src[:, q * kq:(q + 1) * kq, :],
            )

        mask_sb = singles.tile([128, SP], bf16, name="mask_sb")
        nc.sync.dma_start(out=mask_sb, in_=mask)
        ident = singles.tile([128, 128], bf16, name="ident")
        make_identity(nc, ident)
        sigb = singles.tile([128, 1], f32, name="sigb")
        nc.vector.memset(sigb, -inv_lam * CTR)   # sigmoid bias

        # x passthrough in fp16: convert on DVE, one strided DMA out
        x16 = singles.tile([128, ktiles * N], f16, name="x16")
        nc.vector.tensor_copy(out=x16, in_=x_big)
        x16_r = x16.rearrange("p (k n) -> p k n", n=N)
        out_xcols = bass.AP(
            tensor=out.tensor,
            offset=out.offset,
            ap=[[OUTC, 128], [OUTC * 128, ktiles], [1, N]],
        )
        nc.sync.dma_start(out=out_xcols, in_=x16_r)
        if dummy is not None:
            nc.sync.dma_start(out=dummy, in_=x16_r[:, 0, :])

        # t = (1/x - 1)^lam = exp(lam * ln(1/x - 1)).  recip on DVE
        # (approx_fast: ~51 ULP, 1 elem/cycle), ln on ACT (same
        # natural_log set as the bulk pass), exp as a degree-7 Taylor
        # polynomial on DVE (|lam*w| < 0.89) so no extra ACT table set.
        t_big = singles.tile([128, ktiles * N], f32, name="t_big")
        z_big = singles.tile([128, ktiles * N], f32, name="z_big")
        nc.vector.reciprocal_approx_fast(out=t_big, in_=x_big)
        nc.vector.tensor_scalar_add(out=t_big, in0=t_big, scalar1=-1.0)
        tchain_ln = nc.scalar.activation(out=t_big, in_=t_big, func=AF.Ln)
        fact = [1.0, 1.0, 2.0, 6.0, 24.0, 120.0, 720.0, 5040.0]
        A0 = mybir.AluOpType
        nc.vector.tensor_scalar_mul(out=z_big, in0=t_big,
                                    scalar1=float(lam))
        nc.vector.tensor_scalar_mul(out=t_big, in0=z_big,
                                    scalar1=1.0 / fact[7])
        for k in range(6, 0, -1):
            nc.vector.scalar_tensor_tensor(
                out=t_big, in0=t_big, scalar=1.0 / fact[k], in1=z_big,
                op0=A0.add, op1=A0.mult,
            )
        nc.vector.tensor_scalar_add(out=t_big, in0=t_big, scalar1=1.0)

        def _build_w(g):
            stage = stagep.tile([128, 128], bf16, name="stage", tag="stage")
            st_r = stage.rearrange("p (j h) -> p j h", h=32)
            hi = st_r[:, :, 0:16]    # [[32,4],[1,16]] strided dest
            lo = st_r[:, :, 16:32]
            src = t_big[:, g * 4 * N:(g + 1) * 4 * N]
            nc.vector.tensor_copy(out=hi, in_=src)         # f32->bf16
            nc.vector.tensor_sub(out=lo, in0=src, in1=hi)  # residual
            ptr = psum.tile([128, 128], bf16, name="ptr", tag="mm")
            nc.tensor.transpose(ptr, stage, ident)
            W = wp.tile([128, 128], bf16, name="W", tag="W")
            nc.vector.tensor_copy(out=W, in_=ptr)
            return W

        def _dve_h(w_t):
            """DVE pipeline: cols [0:DC] of each of the 4 tiles in w_t
            (fp16 w' = ln(s)-CTR) -> h, written back in place."""
            w_r = w_t.rearrange("p (j c) -> p j c", c=SP)
            wl = w_r[:, :, 0:DC]
            sh = [128, TILES_PER_GROUP * DC]
            ib = dvp.tile(sh, i16, name="ib", tag="ib")
            mf = dvp.tile(sh, i16, name="mf", tag="mf")
            qb = dvp.tile(sh, f16, name="qb", tag="qb")
            A = mybir.AluOpType
            # 1. Schraudolph bits: i16 = cvt(S1*w' + S2)
            nc.vector.tensor_scalar(out=ib, in0=wl, scalar1=SCH_S1,
                                    scalar2=SCH_S2, op0=A.mult, op1=A.add)
            u0 = ib.bitcast(f16)
            # 2. mantissa m = 1+f in [1,2):  (i & 1023) | 0x3C00
            nc.vector.tensor_scalar(out=mf, in0=ib, scalar1=1023,
                                    scalar2=0x3C00, op0=A.bitwise_and,
                                    op1=A.bitwise_or)
            mff = mf.bitcast(f16)
            # 3. q = (m + beta) * m
            nc.vector.scalar_tensor_tensor(out=qb, in0=mff,
                                           scalar=EXP2_BETA, in1=mff,
                                           op0=A.add, op1=A.mult)
            # 4. v = (q + gamma) * u0   (u0 dead after this; mf dead)
            vb = mf.bitcast(f16)
            nc.vector.scalar_tensor_tensor(out=vb, in0=qb,
                                           scalar=EXP2_GAMMA, in1=u0,
                                           op0=A.add, op1=A.mult)
            # 5. d = alpha*v + 1
            db = qb
            nc.vector.tensor_scalar(out=db, in0=vb, scalar1=EXP2_ALPHA,
                                    scalar2=1.0, op0=A.mult, op1=A.add)
            # 6. r0 = bitcast(K2 - bits(d))  (int16 arith, exact in fp32 ALU)
            nc.vector.tensor_scalar(out=ib, in0=db.bitcast(i16),
                                    scalar1=-1, scalar2=RECIP_K2,
                                    op0=A.mult, op1=A.add)
            r0 = ib.bitcast(f16)
            # 7. m1 = d * r0
            m1 = mf.bitcast(f16)
            nc.vector.tensor_tensor(out=m1, in0=db, in1=r0, op=A.mult)
            # 8. m2 = 2' - m1
            nc.vector.tensor_scalar(out=m1, in0=m1, scalar1=-1.0,
                                    scalar2=NEWTON_TWO, op0=A.mult,
                                    op1=A.add)
            # 9. h = m2 * r0  -> back into the w tile columns
            nc.vector.tensor_tensor(out=wl, in0=m1, in1=r0, op=A.mult)

        # The scheduler reorders ACT freely, which ping-pongs the ln and
        # sigmoid table sets (2.66us per reload).  Chain every ACT
        # instruction behind its predecessor in emission order so each
        # phase is strictly [all Lns] -> [all Sigmoids]: 2 loads/phase.
        from concourse.tile_rust import add_dep_helper

        prev_act = [tchain_ln]

        def _act(inst):
            add_dep_helper(inst.ins, prev_act[0].ins, sync=False,
                           reason="pin ACT table-set order")
            prev_act[0] = inst
            return inst

        Ws = {}
        g_next = 0
        for phase_size in PHASES:
            glist = list(range(g_next, g_next + phase_size))
            g_next += phase_size
            ws = {}
            # ln sub-phase (table set: natural_log)
            for g in glist:
                if g not in Ws:
                    Ws[g] = _build_w(g)
                w_t = up.tile([128, TILES_PER_GROUP * SP], f16, name="w",
                              tag="w")
                ws[g] = w_t
                w_r = w_t.rearrange("p (j c) -> p j c", c=SP)
                cs = 0
                for wid in CHUNKS:
                    pm = psum.tile([128, 4 * 512], f32, name="pm", tag="mm")
                    pm_r = pm.rearrange("p (j i) -> p j i", i=512)
                    for j in range(TILES_PER_GROUP):
                        nc.tensor.matmul(
                            pm[:, 512 * j: 512 * j + wid],
                            Ws[g][32 * j: 32 * j + 32, :],
                            mask_sb[32 * j: 32 * j + 32, cs: cs + wid],
                            start=True,
                            stop=True,
                            tile_position=(32 * j, 0),
                        )
                    _act(nc.scalar.activation(
                        out=w_r[:, :, cs: cs + wid], in_=pm_r[:, :, 0:wid],
                        func=AF.Ln, scale=LN_SCALE,
                    ))
                    cs += wid
                _dve_h(w_t)   # DVE path for cols [0:DC] of each tile
            # sigmoid sub-phase (table set B), per tile so DMAs fire asap
            for g in glist:
                w_r = ws[g].rearrange("p (j c) -> p j c", c=SP)
                for j in range(TILES_PER_GROUP):
                    _act(nc.scalar.activation(
                        out=w_r[:, j, DC:SP], in_=w_r[:, j, DC:SP],
                        func=AF.Sigmoid, scale=-float(inv_lam), bias=sigb,
                    ))
                    r0 = (g * TILES_PER_GROUP + j) * 128
                    nc.sync.dma_start(
                        out=out[r0:r0 + 128, N:OUTC],
                        in_=ws[g][:, SP * j: SP * j + S],
                    )


_compiled = {}


def _get_compiled(lam: float, reps: int = 0):
    key = (float(lam), reps)
    if key in _compiled:
        return _compiled[key]
    _ensure_concourse()
    import concourse.tile as tile
    from concourse import bacc, mybir

    nc = bacc.Bacc("TRN2", target_bir_lowering=False, debug=False,
                   enable_asserts=False)
    x_ap = nc.dram_tensor("x", [BC, N], mybir.dt.float32,
                          kind="ExternalInput").ap()
    mask_ap = nc.dram_tensor("mask", [128, SP], mybir.dt.bfloat16,
                             kind="ExternalInput").ap()
    if reps:
        # benchmark variant: big output stays in device DRAM (Internal)
        # so each axon launch doesn't ship 330MB back; tiny dummy output
        out_ap = nc.dram_tensor("outb", [BC, OUTC], mybir.dt.float16,
                                kind="Internal").ap()
        dummy_ap = nc.dram_tensor("out", [128, N], mybir.dt.float16,
                                  kind="ExternalOutput").ap()
    else:
        out_ap = nc.dram_tensor("out", [BC, OUTC], mybir.dt.float16,
                                kind="ExternalOutput").ap()
        dummy_ap = None
    with tile.TileContext(nc) as tc:
        _emit_kernel(tc, x_ap, mask_ap, out_ap, float(lam), reps=reps,
                     dummy=dummy_ap)

    nc.compile()
    _compiled[key] = nc
    return nc


def kernel(x, lam):
    x = np.ascontiguousarray(np.asarray(x), dtype=np.float32)
    lam_f = float(np.asarray(lam))
    assert x.shape == (B, N), x.shape
    nc = _get_compiled(lam_f)
    _ensure_concourse()
    from concourse.bass_utils import run_bass_kernel_spmd

    mask = _build_mask_rep()
    in_maps = [
        {"x": x[c * BC:(c + 1) * BC], "mask": mask}
        for c in range(NCORES)
    ]
    res = run_bass_kernel_spmd(nc, in_maps, core_ids=list(range(NCORES)))
    out = np.concatenate([np.asarray(r["out"]) for r in res.results], axis=0)
    return out.astype(np.float32)


# revision 15
# speedup vs baseline: 1.3623x; 1.3623x over previous
"""Trainium2 Bass kernel for the Dombi t-norm feature-expansion module.

Computation (per reference):
    t = (1/x - 1) ** lam                       # [B, 16]
    s = t @ M.T                                # subset sums, M = binary mask [2500, 16]
    h = 1 / (1 + s ** (1/lam))                 # [B, 2500]
    out = concat([x, h], axis=1)               # [B, 2516]

Strategy (8 NeuronCores, pure data parallel over batch; fp16 output,
upcast to fp32 on the host — well inside the 2e-2 rel-err budget):
  - per core shard of 4096 rows: 8 groups x (4 tiles of 128 rows)
  - t = exp(lam * ln(1/x-1)): DVE reciprocal_approx_fast + ACT Ln + DVE
    Taylor exp; group-0's 64 columns go through a separate fast chain so
    the first matmul starts ~2us in.  hi/lo bf16 split + PE transpose.
  - PE: K=32 matmuls fold hi+lo, 4 strips packed via tile_position
  - h: ACT pass 1 computes w = Ln(scale*s) for ALL columns (scale
    centers w around 0, halving its fp16 quantization error); each
    tile's columns then split between two engines:
      * cols [DC:2500] -> ACT Sigmoid(-1/lam * (w + CTR))
      * cols [0:DC]    -> DVE: u = s^(1/lam) via Schraudolph exp2 with
        a perfect-square mantissa correction (all 4x-rate tensor_scalar
        or 2x-rate tensor_tensor fp16/int16 ops — scalar_tensor_tensor
        would run at 1x), then h = 1/(1+u) via bit-trick seed + Newton.
    DC balances the engines (~0.875 ns/elem ACT vs ~4.07 ns/elem DVE).
  - ln/sigmoid are different ACT table sets -> ACT order is pinned with
    explicit dep edges into [lns][sigmoids] phases: 4 loads/iteration
  - tile pools live OUTSIDE the benchmark For_i loop so iterations
    software-pipeline (no all-engine barrier per rep)
  - output streams out per 128-row tile as fp16 (20.6 MB/core)
"""

import os
import sys
from itertools import combinations

import numpy as np

_REPO_CANDIDATES = ("/opt/trn_rl_repo", "/root/.axon_site/_ro/trn_rl_repo")


def _ensure_concourse():
    try:
        import concourse.bass  # noqa: F401
        return
    except ImportError:
        pass
    for p in _REPO_CANDIDATES:
        if os.path.isdir(p) and p not in sys.path:
            sys.path.insert(0, p)
    import concourse.bass  # noqa: F401


B, N, ADD = 32768, 16, 4
NCORES = 8
BC = B // NCORES            # 4096 rows per core
S = 2500                    # number of subsets (sizes 2..4 of 16)
SP = S                      # per-tile column stride in the w tile
OUTC = N + S                # 2516 output columns
TILES_PER_GROUP = 4         # batch tiles of 128 per PE pass
GROUPS = BC // (128 * TILES_PER_GROUP)   # 8
CHUNKS = (512, 512, 512, 512, 452)       # moving-operand chunk widths
# DVE-path columns per tile, by position within a phase: later groups
# carry more DVE work into the sigmoid sub-phase (when ACT stops
# producing new ln output and DVE would otherwise starve)
DCS = tuple(int(t) for t in
            os.environ.get("DOMBI_DCS", "720,820,940,1072").split(","))
PHASES = tuple(
    int(t) for t in os.environ.get("DOMBI_PHASES", "4,4").split(",")
)
W_BUFS = int(os.environ.get("DOMBI_WBUFS", "5"))

# exp2 mantissa-correction quadratic alpha*(m^2 + beta*m + gamma) ~=
# 2^(m-1)/m on [1,2), rel minimax ~3.5e-3 (numpy fit).  Expressed as a
# perfect square s'*(m + beta/2)^2 + s'*(gamma - beta^2/4) with
# s' = 4*alpha (the 2^-2 folds exactly into the Schraudolph bias).
_EXP2_ALPHA, _EXP2_BETA, _EXP2_GAMMA = 0.23368115, -2.97030264, 6.23478840
EXP2_SQ = float(np.sqrt(4 * _EXP2_ALPHA))          # 0.96681156
EXP2_MHB = EXP2_SQ * _EXP2_BETA / 2                # -1.43586147
EXP2_GPP = 4 * _EXP2_ALPHA * (_EXP2_GAMMA - _EXP2_BETA ** 2 / 4)  # 3.766112
# fp16 reciprocal seed: r0 = bitcast(K2 - bits(d)), ~5.1% over d in
# [1.4, 2050]; one Newton with slight over-relaxation recovers ~1.4e-3
RECIP_K2 = 0x7798
NEWTON_TWO = 2.0013

# instruction-name -> human label, filled at emission (analysis only)
LABELS = {}


def _lbl(inst, label):
    try:
        LABELS[inst.ins.name] = label
    except Exception:
        pass
    return inst


def _build_mask_rep():
    """[128, 2500] bf16: M.T tiled 8x vertically (4 strips x {hi, lo})."""
    import ml_dtypes
    rows = []
    for i in range(2, ADD + 1):
        for c in combinations(range(N), i):
            r = np.zeros(N, dtype=np.float32)
            r[list(c)] = 1.0
            rows.append(r)
    M = np.stack(rows)                       # [2500, 16]
    MT = M.T.astype(np.float32)              # [16, 2500]
    rep = np.zeros((128, S), dtype=np.float32)
    for j in range(TILES_PER_GROUP):
        rep[32 * j: 32 * j + 16] = MT        # hi half of the strip
        rep[32 * j + 16: 32 * j + 32] = MT   # lo half of the strip
    return rep.astype(ml_dtypes.bfloat16)


def _emit_kernel(tc, x, mask, out, lam, reps=0, dummy=None):
    import concourse.bass as bass  # noqa: F401
    from concourse import mybir
    from concourse.tile_rust import add_dep_helper
    from contextlib import ExitStack

    nc = tc.nc
    f32 = mybir.dt.float32
    f16 = mybir.dt.float16
    i16 = mybir.dt.int16
    bf16 = mybir.dt.bfloat16
    AF = mybir.ActivationFunctionType
    A = mybir.AluOpType
    inv_lam = 1.0 / lam                      # 10/3 for lam=0.3
    c2 = inv_lam / float(np.log(2.0))        # base-2 exponent multiplier

    # center of the ln(s) range (s = sums of 2..4 terms of (1/x-1)^lam,
    # x in (0.05, 0.95)) -- the Ln pass computes w' = ln(s) - CTR so the
    # fp16 w' sits near 0 (half the quantization error of raw ln s)
    vmin, vmax = 1.0 / 0.95 - 1.0, 1.0 / 0.05 - 1.0
    wlo, whi = vmin ** lam, vmax ** lam
    CTR = 0.5 * (float(np.log(2 * wlo)) + float(np.log(ADD * whi)))
    LN_SCALE = float(np.exp(-CTR))

    # Schraudolph step: i16 = round(S1*w' + S2) gives fp16 bits of
    # ~2^(c2*(w'+CTR) - 2) = s^(1/lam) / 4  (the -2048 pre-applies the
    # 2^-2 factored out of the correction quadratic)
    SCH_S1 = 1024.0 * c2
    SCH_S2 = 1024.0 * (15.0 + c2 * CTR) - 2048.0

    ktiles = BC // 128                      # 32 batch tiles of 128 rows
    with ExitStack() as ctx:
        singles = ctx.enter_context(tc.tile_pool(name="singles", bufs=2))
        stagep = ctx.enter_context(tc.tile_pool(name="stagep", bufs=4))
        wp = ctx.enter_context(tc.tile_pool(name="wp", bufs=GROUPS))
        up = ctx.enter_context(tc.tile_pool(name="up", bufs=W_BUFS))
        dvp = ctx.enter_context(tc.tile_pool(name="dvp", bufs=2))
        psum = ctx.enter_context(
            tc.tile_pool(name="psum", bufs=2, space="PSUM"))

        # loop-invariant constants live outside the rep loop
        sigb = singles.tile([128, 1], f32, name="sigb", tag="sigb")
        nc.vector.memset(sigb, -inv_lam * CTR)   # sigmoid bias

        def body():
            # whole x shard resident in SBUF: x_big[p,16k+n] = x[128k+p,n]
            x_big = singles.tile([128, ktiles * N], f32, name="x_big",
                                 tag="x_big")
            xb_r = x_big.rearrange("p (k n) -> p k n", n=N)
            x_src = x.rearrange("(k p) n -> p k n", p=128)
            kq = ktiles // 4
            for q in range(4):
                nc.sync.dma_start(
                    out=xb_r[:, q * kq:(q + 1) * kq, :],
                    in_=x_src[:, q * kq:(q + 1) * kq, :],
                )

            mask_sb = singles.tile([128, SP], bf16, name="mask_sb",
                                   tag="mask")
            nc.sync.dma_start(out=mask_sb, in_=mask)

            # x passthrough in fp16: convert on DVE, strided DMA out
            x16 = singles.tile([128, ktiles * N], f16, name="x16",
                               tag="x16")
            nc.vector.tensor_copy(out=x16, in_=x_big)
            x16_r = x16.rearrange("p (k n) -> p k n", n=N)
            out_xcols = bass.AP(
                tensor=out.tensor,
                offset=out.offset,
                ap=[[OUTC, 128], [OUTC * 128, ktiles], [1, N]],
            )
            nc.sync.dma_start(out=out_xcols, in_=x16_r)
            if dummy is not None:
                nc.sync.dma_start(out=dummy, in_=x16_r[:, 0, :])

            # t = (1/x-1)^lam = exp(lam*ln(1/x-1)): recip_approx_fast on
            # DVE, Ln on ACT (same natural_log set as the bulk pass), exp
            # as a degree-7 DVE Taylor (|lam*w| < 0.89).  Group 0's 64
            # columns run as a separate fast chain so W0 and the first
            # matmuls are ready ~2us in; the bulk chain is pinned behind
            # it so the scheduler doesn't stretch the critical path.
            t_big = singles.tile([128, ktiles * N], f32, name="t_big",
                                 tag="t_big")
            z_big = singles.tile([128, ktiles * N], f32, name="z_big",
                                 tag="z_big")
            fact = [1.0, 1.0, 2.0, 6.0, 24.0, 120.0, 720.0, 5040.0]

            act_chain = [None]

            def _act(inst):
                if act_chain[0] is not None:
                    add_dep_helper(inst.ins, act_chain[0].ins, sync=False,
                                   reason="pin ACT table-set order")
                act_chain[0] = inst
                return inst

            def _t_chain(sl):
                nc.vector.reciprocal_approx_fast(out=t_big[:, sl],
                                                 in_=x_big[:, sl])
                head = nc.vector.tensor_scalar_add(out=t_big[:, sl],
                                                   in0=t_big[:, sl],
                                                   scalar1=-1.0)
                _act(nc.scalar.activation(out=t_big[:, sl],
                                          in_=t_big[:, sl], func=AF.Ln))
                nc.vector.tensor_scalar_mul(out=z_big[:, sl],
                                            in0=t_big[:, sl],
                                            scalar1=float(lam))
                nc.vector.tensor_scalar_mul(out=t_big[:, sl],
                                            in0=z_big[:, sl],
                                            scalar1=1.0 / fact[7])
                tail = None
                for k in range(6, 0, -1):
                    tail = nc.vector.scalar_tensor_tensor(
                        out=t_big[:, sl], in0=t_big[:, sl],
                        scalar=1.0 / fact[k], in1=z_big[:, sl],
                        op0=A.add, op1=A.mult,
                    )
                tail = nc.vector.tensor_scalar_add(out=t_big[:, sl],
                                                   in0=t_big[:, sl],
                                                   scalar1=1.0)
                return head, tail

            _, fast_tail = _t_chain(slice(0, 4 * N))
            bulk_head, _ = _t_chain(slice(4 * N, ktiles * N))
            add_dep_helper(bulk_head.ins, fast_tail.ins, sync=False,
                           reason="stagger bulk t-chain")

            def _build_w(g):
                # hi/lo bf16 split staged on DVE, then transposed via the
                # DMA crossbar -- no PE/PSUM/DVE-copy in the W path, so
                # W-builds slot into DVE idle windows
                stage = stagep.tile([128, 128], bf16, name="stage",
                                    tag="stage")
                st_r = stage.rearrange("p (j h) -> p j h", h=32)
                hi = st_r[:, :, 0:16]
                lo = st_r[:, :, 16:32]
                src = t_big[:, g * 4 * N:(g + 1) * 4 * N]
                nc.vector.tensor_copy(out=hi, in_=src)         # f32->bf16
                nc.vector.tensor_sub(out=lo, in0=src, in1=hi)  # residual
                W = wp.tile([128, 128], bf16, name="W", tag="W")
                nc.sync.dma_start_transpose(out=W, in_=stage)
                return W

            def _dve_h(w_t, gi, dc):
                """DVE: cols [0:dc] of each tile in w_t -> h, in place."""
                w_r = w_t.rearrange("p (j c) -> p j c", c=SP)
                wl = w_r[:, :, 0:dc]
                sh = [128, TILES_PER_GROUP * dc]
                ib = dvp.tile(sh, i16, name="ib", tag="ib")
                mf = dvp.tile(sh, i16, name="mf", tag="mf")
                qb = dvp.tile(sh, f16, name="qb", tag="qb")
                # 1. Schraudolph bits: i16 = cvt(S1*w' + S2)
                _lbl(nc.vector.tensor_scalar(out=ib, in0=wl,
                                             scalar1=SCH_S1,
                                             scalar2=SCH_S2, op0=A.mult,
                                             op1=A.add), f"dve{gi}.1sch")
                u0 = ib.bitcast(f16)
                # 2. mantissa m = 1+f in [1,2): (i & 1023) | 0x3C00
                _lbl(nc.vector.tensor_scalar(out=mf, in0=ib, scalar1=1023,
                                             scalar2=0x3C00,
                                             op0=A.bitwise_and,
                                             op1=A.bitwise_or),
                     f"dve{gi}.2mask")
                mff = mf.bitcast(f16)
                # 3. mh = SQ*m + MHB
                _lbl(nc.vector.tensor_scalar(out=qb, in0=mff,
                                             scalar1=EXP2_SQ,
                                             scalar2=EXP2_MHB, op0=A.mult,
                                             op1=A.add), f"dve{gi}.3mh")
                # 4. p = mh^2
                pb = mf.bitcast(f16)
                _lbl(nc.vector.tensor_tensor(out=pb, in0=qb, in1=qb,
                                             op=A.mult), f"dve{gi}.4p")
                # 5. pp = p + GPP
                _lbl(nc.vector.tensor_scalar_add(out=qb, in0=pb,
                                                 scalar1=EXP2_GPP),
                     f"dve{gi}.5pp")
                # 6. v = pp * u0  =  s^(1/lam)
                vb = mf.bitcast(f16)
                _lbl(nc.vector.tensor_tensor(out=vb, in0=qb, in1=u0,
                                             op=A.mult), f"dve{gi}.6v")
                # 7. d = v + 1
                db = qb
                _lbl(nc.vector.tensor_scalar_add(out=db, in0=vb,
                                                 scalar1=1.0),
                     f"dve{gi}.7d")
                # 8. r0 = bitcast(K2 - bits(d))  (int16 arith)
                _lbl(nc.vector.tensor_scalar(out=ib, in0=db.bitcast(i16),
                                             scalar1=-1, scalar2=RECIP_K2,
                                             op0=A.mult, op1=A.add),
                     f"dve{gi}.8r0")
                r0 = ib.bitcast(f16)
                # 9. m1 = d * r0
                m1 = mf.bitcast(f16)
                _lbl(nc.vector.tensor_tensor(out=m1, in0=db, in1=r0,
                                             op=A.mult), f"dve{gi}.9m1")
                # 10. m2 = 2' - m1
                _lbl(nc.vector.tensor_scalar(out=m1, in0=m1, scalar1=-1.0,
                                             scalar2=NEWTON_TWO,
                                             op0=A.mult, op1=A.add),
                     f"dve{gi}.10m2")
                # 11. h = m2 * r0  -> back into the w tile columns
                _lbl(nc.vector.tensor_tensor(out=wl, in0=m1, in1=r0,
                                             op=A.mult), f"dve{gi}.11h")

            Ws = {}
            phase_groups = []
            g_next = 0
            for phase_size in PHASES:
                phase_groups.append(list(range(g_next, g_next + phase_size)))
                g_next += phase_size
            dc_of = {}
            for glist in phase_groups:
                for k, g in enumerate(glist):
                    dc_of[g] = DCS[k * len(DCS) // len(glist)]
            for g in phase_groups[0]:
                Ws[g] = _build_w(g)
            for pi, glist in enumerate(phase_groups):
                ws = {}
                # ln sub-phase (table set: natural_log)
                for g in glist:
                    w_t = up.tile([128, TILES_PER_GROUP * SP], f16,
                                  name="w", tag="w")
                    ws[g] = w_t
                    w_r = w_t.rearrange("p (j c) -> p j c", c=SP)
                    cs = 0
                    for wid in CHUNKS:
                        pm = psum.tile([128, 4 * 512], f32, name="pm",
                                       tag="mm")
                        pm_r = pm.rearrange("p (j i) -> p j i", i=512)
                        for j in range(TILES_PER_GROUP):
                            nc.tensor.matmul(
                                pm[:, 512 * j: 512 * j + wid],
                                Ws[g][32 * j: 32 * j + 32, :],
                                mask_sb[32 * j: 32 * j + 32, cs: cs + wid],
                                start=True,
                                stop=True,
                                tile_position=(32 * j, 0),
                            )
                        _lbl(_act(nc.scalar.activation(
                            out=w_r[:, :, cs: cs + wid],
                            in_=pm_r[:, :, 0:wid],
                            func=AF.Ln, scale=LN_SCALE,
                        )), f"ln{g}.{cs}")
                        cs += wid
                    _dve_h(w_t, g, dc_of[g])
                # W-builds for the NEXT phase: DVE hi/lo staging + DMA
                # transposes run while ACT is busy with the sigmoids, so
                # the next phase's matmuls/lns start without waiting
                if pi + 1 < len(phase_groups):
                    for g2 in phase_groups[pi + 1]:
                        Ws[g2] = _build_w(g2)
                # sigmoid sub-phase (other table set): one op per group,
                # then the tile DMAs
                for g in glist:
                    w_r = ws[g].rearrange("p (j c) -> p j c", c=SP)
                    _lbl(_act(nc.scalar.activation(
                        out=w_r[:, :, dc_of[g]:SP],
                        in_=w_r[:, :, dc_of[g]:SP],
                        func=AF.Sigmoid, scale=-float(inv_lam), bias=sigb,
                    )), f"sig{g}")
                    for j in range(TILES_PER_GROUP):
                        r0 = (g * TILES_PER_GROUP + j) * 128
                        # issue from the idle GPSIMD queue -- the SP
                        # sequencer's ~1.7us/DMA issue cost serializes
                        # against the input loads otherwise
                        nc.gpsimd.dma_start(
                            out=out[r0:r0 + 128, N:OUTC],
                            in_=ws[g][:, SP * j: SP * j + S],
                        )

        if reps:
            unroll = 4 if reps % 4 == 0 else 1
            with tc.For_i(0, reps // unroll, 1):
                for _ in range(unroll):
                    body()
        else:
            body()


_compiled = {}


def _get_compiled(lam: float, reps: int = 0):
    key = (float(lam), reps)
    if key in _compiled:
        return _compiled[key]
    _ensure_concourse()
    import concourse.tile as tile
    from concourse import bacc, mybir

    nc = bacc.Bacc("TRN2", target_bir_lowering=False, debug=False,
                   enable_asserts=False)
    x_ap = nc.dram_tensor("x", [BC, N], mybir.dt.float32,
                          kind="ExternalInput").ap()
    mask_ap = nc.dram_tensor("mask", [128, SP], mybir.dt.bfloat16,
                             kind="ExternalInput").ap()
    if reps:
        # benchmark variant: big output stays in device DRAM (Internal)
        # so each axon launch doesn't ship 330MB back; tiny dummy output
        out_ap = nc.dram_tensor("outb", [BC, OUTC], mybir.dt.float16,
                                kind="Internal").ap()
        dummy_ap = nc.dram_tensor("out", [128, N], mybir.dt.float16,
                                  kind="ExternalOutput").ap()
    else:
        out_ap = nc.dram_tensor("out", [BC, OUTC], mybir.dt.float16,
                                kind="ExternalOutput").ap()
        dummy_ap = None
    with tile.TileContext(nc) as tc:
        _emit_kernel(tc, x_ap, mask_ap, out_ap, float(lam), reps=reps,
                     dummy=dummy_ap)

    nc.compile()
    _compiled[key] = nc
    return nc


def kernel(x, lam):
    x = np.ascontiguousarray(np.asarray(x), dtype=np.float32)
    lam_f = float(np.asarray(lam))
    assert x.shape == (B, N), x.shape
    nc = _get_compiled(lam_f)
    _ensure_concourse()
    from concourse.bass_utils import run_bass_kernel_spmd

    mask = _build_mask_rep()
    in_maps = [
        {"x": x[c * BC:(c + 1) * BC], "mask": mask}
        for c in range(NCORES)
    ]
    res = run_bass_kernel_spmd(nc, in_maps, core_ids=list(range(NCORES)))
    out = np.concatenate([np.asarray(r["out"]) for r in res.results], axis=0)
    return out.astype(np.float32)


# revision 17
# speedup vs baseline: 1.3678x; 1.0040x over previous
"""Trainium2 Bass kernel for the Dombi t-norm feature-expansion module.

Computation (per reference):
    t = (1/x - 1) ** lam                       # [B, 16]
    s = t @ M.T                                # subset sums, M = binary mask [2500, 16]
    h = 1 / (1 + s ** (1/lam))                 # [B, 2500]
    out = concat([x, h], axis=1)               # [B, 2516]

Strategy (8 NeuronCores, pure data parallel over batch; fp16 output,
upcast to fp32 on the host — well inside the 2e-2 rel-err budget):
  - per core shard of 4096 rows: 8 groups x (4 tiles of 128 rows)
  - t = exp(lam * ln(1/x-1)): DVE reciprocal_approx_fast + ACT Ln + DVE
    Taylor exp; group-0's 64 columns go through a separate fast chain so
    the first matmul starts ~2us in.  hi/lo bf16 split + PE transpose.
  - PE: K=32 matmuls fold hi+lo, 4 strips packed via tile_position
  - h: ACT pass 1 computes w = Ln(scale*s) for ALL columns (scale
    centers w around 0, halving its fp16 quantization error); each
    tile's columns then split between two engines:
      * cols [DC:2500] -> ACT Sigmoid(-1/lam * (w + CTR))
      * cols [0:DC]    -> DVE: u = s^(1/lam) via Schraudolph exp2 with
        a perfect-square mantissa correction (all 4x-rate tensor_scalar
        or 2x-rate tensor_tensor fp16/int16 ops — scalar_tensor_tensor
        would run at 1x), then h = 1/(1+u) via bit-trick seed + Newton.
    DC balances the engines (~0.875 ns/elem ACT vs ~4.07 ns/elem DVE).
  - ln/sigmoid are different ACT table sets -> ACT order is pinned with
    explicit dep edges into [lns][sigmoids] phases: 4 loads/iteration
  - tile pools live OUTSIDE the benchmark For_i loop so iterations
    software-pipeline (no all-engine barrier per rep)
  - output streams out per 128-row tile as fp16 (20.6 MB/core)
"""

import os
import sys
from itertools import combinations

import numpy as np

_REPO_CANDIDATES = ("/opt/trn_rl_repo", "/root/.axon_site/_ro/trn_rl_repo")


def _ensure_concourse():
    try:
        import concourse.bass  # noqa: F401
        return
    except ImportError:
        pass
    for p in _REPO_CANDIDATES:
        if os.path.isdir(p) and p not in sys.path:
            sys.path.insert(0, p)
    import concourse.bass  # noqa: F401


B, N, ADD = 32768, 16, 4
NCORES = 8
BC = B // NCORES            # 4096 rows per core
S = 2500                    # number of subsets (sizes 2..4 of 16)
SP = S                      # per-tile column stride in the w tile
OUTC = N + S                # 2516 output columns
TILES_PER_GROUP = 4         # batch tiles of 128 per PE pass
GROUPS = BC // (128 * TILES_PER_GROUP)   # 8
CHUNKS = (512, 512, 512, 512, 452)       # moving-operand chunk widths
# DVE-path columns per tile, by position within a phase: later groups
# carry more DVE work into the sigmoid sub-phase (when ACT stops
# producing new ln output and DVE would otherwise starve)
DCS = tuple(int(t) for t in
            os.environ.get("DOMBI_DCS", "760,860,980,1108").split(","))
PHASES = tuple(
    int(t) for t in os.environ.get("DOMBI_PHASES", "4,4").split(",")
)
W_BUFS = int(os.environ.get("DOMBI_WBUFS", "5"))

# exp2 mantissa-correction quadratic alpha*(m^2 + beta*m + gamma) ~=
# 2^(m-1)/m on [1,2), rel minimax ~3.5e-3 (numpy fit).  Expressed as a
# perfect square s'*(m + beta/2)^2 + s'*(gamma - beta^2/4) with
# s' = 4*alpha (the 2^-2 folds exactly into the Schraudolph bias).
_EXP2_ALPHA, _EXP2_BETA, _EXP2_GAMMA = 0.23368115, -2.97030264, 6.23478840
EXP2_SQ = float(np.sqrt(4 * _EXP2_ALPHA))          # 0.96681156
EXP2_MHB = EXP2_SQ * _EXP2_BETA / 2                # -1.43586147
EXP2_GPP = 4 * _EXP2_ALPHA * (_EXP2_GAMMA - _EXP2_BETA ** 2 / 4)  # 3.766112
# fp16 reciprocal seed: r0 = bitcast(K2 - bits(d)), ~5.1% over d in
# [1.4, 2050]; one Newton with slight over-relaxation recovers ~1.4e-3
RECIP_K2 = 0x7798
NEWTON_TWO = 2.0013

# instruction-name -> human label, filled at emission (analysis only)
LABELS = {}


def _lbl(inst, label):
    try:
        LABELS[inst.ins.name] = label
    except Exception:
        pass
    return inst


def _build_mask_rep():
    """[128, 2500] bf16: M.T tiled 8x vertically (4 strips x {hi, lo})."""
    import ml_dtypes
    rows = []
    for i in range(2, ADD + 1):
        for c in combinations(range(N), i):
            r = np.zeros(N, dtype=np.float32)
            r[list(c)] = 1.0
            rows.append(r)
    M = np.stack(rows)                       # [2500, 16]
    MT = M.T.astype(np.float32)              # [16, 2500]
    rep = np.zeros((128, S), dtype=np.float32)
    for j in range(TILES_PER_GROUP):
        rep[32 * j: 32 * j + 16] = MT        # hi half of the strip
        rep[32 * j + 16: 32 * j + 32] = MT   # lo half of the strip
    return rep.astype(ml_dtypes.bfloat16)


def _emit_kernel(tc, x, mask, out, lam, reps=0, dummy=None):
    import concourse.bass as bass  # noqa: F401
    from concourse import mybir
    from concourse.tile_rust import add_dep_helper
    from contextlib import ExitStack

    nc = tc.nc
    f32 = mybir.dt.float32
    f16 = mybir.dt.float16
    i16 = mybir.dt.int16
    bf16 = mybir.dt.bfloat16
    AF = mybir.ActivationFunctionType
    A = mybir.AluOpType
    inv_lam = 1.0 / lam                      # 10/3 for lam=0.3
    c2 = inv_lam / float(np.log(2.0))        # base-2 exponent multiplier

    # center of the ln(s) range (s = sums of 2..4 terms of (1/x-1)^lam,
    # x in (0.05, 0.95)) -- the Ln pass computes w' = ln(s) - CTR so the
    # fp16 w' sits near 0 (half the quantization error of raw ln s)
    vmin, vmax = 1.0 / 0.95 - 1.0, 1.0 / 0.05 - 1.0
    wlo, whi = vmin ** lam, vmax ** lam
    CTR = 0.5 * (float(np.log(2 * wlo)) + float(np.log(ADD * whi)))
    LN_SCALE = float(np.exp(-CTR))

    # Schraudolph step: i16 = round(S1*w' + S2) gives fp16 bits of
    # ~2^(c2*(w'+CTR) - 2) = s^(1/lam) / 4  (the -2048 pre-applies the
    # 2^-2 factored out of the correction quadratic)
    SCH_S1 = 1024.0 * c2
    SCH_S2 = 1024.0 * (15.0 + c2 * CTR) - 2048.0

    ktiles = BC // 128                      # 32 batch tiles of 128 rows
    with ExitStack() as ctx:
        singles = ctx.enter_context(tc.tile_pool(name="singles", bufs=2))
        stagep = ctx.enter_context(tc.tile_pool(name="stagep", bufs=4))
        wp = ctx.enter_context(tc.tile_pool(name="wp", bufs=GROUPS))
        up = ctx.enter_context(tc.tile_pool(name="up", bufs=W_BUFS))
        dvp = ctx.enter_context(tc.tile_pool(name="dvp", bufs=2))
        psum = ctx.enter_context(
            tc.tile_pool(name="psum", bufs=2, space="PSUM"))

        # loop-invariant constants live outside the rep loop
        sigb = singles.tile([128, 1], f32, name="sigb", tag="sigb")
        nc.vector.memset(sigb, -inv_lam * CTR)   # sigmoid bias

        def body():
            # whole x shard resident in SBUF: x_big[p,16k+n] = x[128k+p,n]
            x_big = singles.tile([128, ktiles * N], f32, name="x_big",
                                 tag="x_big")
            xb_r = x_big.rearrange("p (k n) -> p k n", n=N)
            x_src = x.rearrange("(k p) n -> p k n", p=128)
            kq = ktiles // 4
            for q in range(4):
                nc.sync.dma_start(
                    out=xb_r[:, q * kq:(q + 1) * kq, :],
                    in_=x_src[:, q * kq:(q + 1) * kq, :],
                )

            mask_sb = singles.tile([128, SP], bf16, name="mask_sb",
                                   tag="mask")
            nc.sync.dma_start(out=mask_sb, in_=mask)

            # x passthrough in fp16: convert on DVE, strided DMA out
            x16 = singles.tile([128, ktiles * N], f16, name="x16",
                               tag="x16")
            nc.vector.tensor_copy(out=x16, in_=x_big)
            x16_r = x16.rearrange("p (k n) -> p k n", n=N)
            out_xcols = bass.AP(
                tensor=out.tensor,
                offset=out.offset,
                ap=[[OUTC, 128], [OUTC * 128, ktiles], [1, N]],
            )
            nc.sync.dma_start(out=out_xcols, in_=x16_r)
            if dummy is not None:
                nc.sync.dma_start(out=dummy, in_=x16_r[:, 0, :])

            # t = (1/x-1)^lam = exp(lam*ln(1/x-1)): recip_approx_fast on
            # DVE, Ln on ACT (same natural_log set as the bulk pass), exp
            # as a degree-7 DVE Taylor (|lam*w| < 0.89).  Group 0's 64
            # columns run as a separate fast chain so W0 and the first
            # matmuls are ready ~2us in; the bulk chain is pinned behind
            # it so the scheduler doesn't stretch the critical path.
            t_big = singles.tile([128, ktiles * N], f32, name="t_big",
                                 tag="t_big")
            z_big = singles.tile([128, ktiles * N], f32, name="z_big",
                                 tag="z_big")
            fact = [1.0, 1.0, 2.0, 6.0, 24.0, 120.0, 720.0, 5040.0]

            act_chain = [None]

            def _act(inst):
                if act_chain[0] is not None:
                    add_dep_helper(inst.ins, act_chain[0].ins, sync=False,
                                   reason="pin ACT table-set order")
                act_chain[0] = inst
                return inst

            def _t_chain(sl):
                nc.vector.reciprocal_approx_fast(out=t_big[:, sl],
                                                 in_=x_big[:, sl])
                head = nc.vector.tensor_scalar_add(out=t_big[:, sl],
                                                   in0=t_big[:, sl],
                                                   scalar1=-1.0)
                _act(nc.scalar.activation(out=t_big[:, sl],
                                          in_=t_big[:, sl], func=AF.Ln))
                nc.vector.tensor_scalar_mul(out=z_big[:, sl],
                                            in0=t_big[:, sl],
                                            scalar1=float(lam))
                nc.vector.tensor_scalar_mul(out=t_big[:, sl],
                                            in0=z_big[:, sl],
                                            scalar1=1.0 / fact[7])
                tail = None
                for k in range(6, 0, -1):
                    tail = nc.vector.scalar_tensor_tensor(
                        out=t_big[:, sl], in0=t_big[:, sl],
                        scalar=1.0 / fact[k], in1=z_big[:, sl],
                        op0=A.add, op1=A.mult,
                    )
                tail = nc.vector.tensor_scalar_add(out=t_big[:, sl],
                                                   in0=t_big[:, sl],
                                                   scalar1=1.0)
                return head, tail

            _, fast_tail = _t_chain(slice(0, 4 * N))
            bulk_head, _ = _t_chain(slice(4 * N, ktiles * N))
            add_dep_helper(bulk_head.ins, fast_tail.ins, sync=False,
                           reason="stagger bulk t-chain")

            def _build_w(g):
                # hi/lo bf16 split staged on DVE, then transposed via the
                # DMA crossbar -- no PE/PSUM/DVE-copy in the W path, so
                # W-builds slot into DVE idle windows
                stage = stagep.tile([128, 128], bf16, name="stage",
                                    tag="stage")
                st_r = stage.rearrange("p (j h) -> p j h", h=32)
                hi = st_r[:, :, 0:16]
                lo = st_r[:, :, 16:32]
                src = t_big[:, g * 4 * N:(g + 1) * 4 * N]
                nc.vector.tensor_copy(out=hi, in_=src)         # f32->bf16
                nc.vector.tensor_sub(out=lo, in0=src, in1=hi)  # residual
                W = wp.tile([128, 128], bf16, name="W", tag="W")
                nc.sync.dma_start_transpose(out=W, in_=stage)
                return W

            def _dve_h(w_t, gi, dc):
                """DVE: cols [0:dc] of each tile in w_t -> h, in place."""
                w_r = w_t.rearrange("p (j c) -> p j c", c=SP)
                wl = w_r[:, :, 0:dc]
                sh = [128, TILES_PER_GROUP * dc]
                ib = dvp.tile(sh, i16, name="ib", tag="ib")
                mf = dvp.tile(sh, i16, name="mf", tag="mf")
                qb = dvp.tile(sh, f16, name="qb", tag="qb")
                # 1. Schraudolph bits: i16 = cvt(S1*w' + S2)
                _lbl(nc.vector.tensor_scalar(out=ib, in0=wl,
                                             scalar1=SCH_S1,
                                             scalar2=SCH_S2, op0=A.mult,
                                             op1=A.add), f"dve{gi}.1sch")
                u0 = ib.bitcast(f16)
                # 2. mantissa m = 1+f in [1,2): (i & 1023) | 0x3C00
                _lbl(nc.vector.tensor_scalar(out=mf, in0=ib, scalar1=1023,
                                             scalar2=0x3C00,
                                             op0=A.bitwise_and,
                                             op1=A.bitwise_or),
                     f"dve{gi}.2mask")
                mff = mf.bitcast(f16)
                # 3. mh = SQ*m + MHB
                _lbl(nc.vector.tensor_scalar(out=qb, in0=mff,
                                             scalar1=EXP2_SQ,
                                             scalar2=EXP2_MHB, op0=A.mult,
                                             op1=A.add), f"dve{gi}.3mh")
                # 4. p = mh^2
                pb = mf.bitcast(f16)
                _lbl(nc.vector.tensor_tensor(out=pb, in0=qb, in1=qb,
                                             op=A.mult), f"dve{gi}.4p")
                # 5. pp = p + GPP
                _lbl(nc.vector.tensor_scalar_add(out=qb, in0=pb,
                                                 scalar1=EXP2_GPP),
                     f"dve{gi}.5pp")
                # 6. v = pp * u0  =  s^(1/lam)
                vb = mf.bitcast(f16)
                _lbl(nc.vector.tensor_tensor(out=vb, in0=qb, in1=u0,
                                             op=A.mult), f"dve{gi}.6v")
                # 7. d = v + 1
                db = qb
                _lbl(nc.vector.tensor_scalar_add(out=db, in0=vb,
                                                 scalar1=1.0),
                     f"dve{gi}.7d")
                # 8. r0 = bitcast(K2 - bits(d))  (int16 arith)
                _lbl(nc.vector.tensor_scalar(out=ib, in0=db.bitcast(i16),
                                             scalar1=-1, scalar2=RECIP_K2,
                                             op0=A.mult, op1=A.add),
                     f"dve{gi}.8r0")
                r0 = ib.bitcast(f16)
                # 9. h = (2' - d*r0)*r0 -> w tile columns; one Newton
                # step as a single pre-registered custom DVE op
                from concourse.dve_ops import RECIPROCAL_APPROX_NR
                _lbl(nc.vector._custom_dve(RECIPROCAL_APPROX_NR, out=wl,
                                           in0=db, in1=r0,
                                           s0=NEWTON_TWO),
                     f"dve{gi}.9nr")

            Ws = {}
            phase_groups = []
            g_next = 0
            for phase_size in PHASES:
                phase_groups.append(list(range(g_next, g_next + phase_size)))
                g_next += phase_size
            dc_of = {}
            for glist in phase_groups:
                for k, g in enumerate(glist):
                    dc_of[g] = DCS[k * len(DCS) // len(glist)]
            for g in phase_groups[0]:
                Ws[g] = _build_w(g)
            for pi, glist in enumerate(phase_groups):
                ws = {}
                # ln sub-phase (table set: natural_log)
                for g in glist:
                    w_t = up.tile([128, TILES_PER_GROUP * SP], f16,
                                  name="w", tag="w")
                    ws[g] = w_t
                    w_r = w_t.rearrange("p (j c) -> p j c", c=SP)
                    cs = 0
                    for wid in CHUNKS:
                        pm = psum.tile([128, 4 * 512], f32, name="pm",
                                       tag="mm")
                        pm_r = pm.rearrange("p (j i) -> p j i", i=512)
                        for j in range(TILES_PER_GROUP):
                            nc.tensor.matmul(
                                pm[:, 512 * j: 512 * j + wid],
                                Ws[g][32 * j: 32 * j + 32, :],
                                mask_sb[32 * j: 32 * j + 32, cs: cs + wid],
                                start=True,
                                stop=True,
                                tile_position=(32 * j, 0),
                            )
                        _lbl(_act(nc.scalar.activation(
                            out=w_r[:, :, cs: cs + wid],
                            in_=pm_r[:, :, 0:wid],
                            func=AF.Ln, scale=LN_SCALE,
                        )), f"ln{g}.{cs}")
                        cs += wid
                    _dve_h(w_t, g, dc_of[g])
                # W-builds for the NEXT phase: DVE hi/lo staging + DMA
                # transposes run while ACT is busy with the sigmoids, so
                # the next phase's matmuls/lns start without waiting
                if pi + 1 < len(phase_groups):
                    for g2 in phase_groups[pi + 1]:
                        Ws[g2] = _build_w(g2)
                # sigmoid sub-phase (other table set): one op per group,
                # then the tile DMAs
                for g in glist:
                    w_r = ws[g].rearrange("p (j c) -> p j c", c=SP)
                    _lbl(_act(nc.scalar.activation(
                        out=w_r[:, :, dc_of[g]:SP],
                        in_=w_r[:, :, dc_of[g]:SP],
                        func=AF.Sigmoid, scale=-float(inv_lam), bias=sigb,
                    )), f"sig{g}")
                    for j in range(TILES_PER_GROUP):
                        r0 = (g * TILES_PER_GROUP + j) * 128
                        # issue from the idle GPSIMD queue -- the SP
                        # sequencer's ~1.7us/DMA issue cost serializes
                        # against the input loads otherwise
                        nc.gpsimd.dma_start(
                            out=out[r0:r0 + 128, N:OUTC],
                            in_=ws[g][:, SP * j: SP * j + S],
                        )

        if reps:
            unroll = 4 if reps % 4 == 0 else 1
            with tc.For_i(0, reps // unroll, 1):
                for _ in range(unroll):
                    body()
        else:
            body()


_compiled = {}


def _get_compiled(lam: float, reps: int = 0):
    key = (float(lam), reps)
    if key in _compiled:
        return _compiled[key]
    _ensure_concourse()
    import concourse.tile as tile
    from concourse import bacc, mybir

    nc = bacc.Bacc("TRN2", target_bir_lowering=False, debug=False,
                   enable_asserts=False)
    x_ap = nc.dram_tensor("x", [BC, N], mybir.dt.float32,
                          kind="ExternalInput").ap()
    mask_ap = nc.dram_tensor("mask", [128, SP], mybir.dt.bfloat16,
                             kind="ExternalInput").ap()
    if reps:
        # benchmark variant: big output stays in device DRAM (Internal)
        # so each axon launch doesn't ship 330MB back; tiny dummy output
        out_ap = nc.dram_tensor("outb", [BC, OUTC], mybir.dt.float16,
                                kind="Internal").ap()
        dummy_ap = nc.dram_tensor("out", [128, N], mybir.dt.float16,
                                  kind="ExternalOutput").ap()
    else:
        out_ap = nc.dram_tensor("out", [BC, OUTC], mybir.dt.float16,
                                kind="ExternalOutput").ap()
        dummy_ap = None
    with tile.TileContext(nc) as tc:
        _emit_kernel(tc, x_ap, mask_ap, out_ap, float(lam), reps=reps,
                     dummy=dummy_ap)

    nc.compile()
    _compiled[key] = nc
    return nc


def kernel(x, lam):
    x = np.ascontiguousarray(np.asarray(x), dtype=np.float32)
    lam_f = float(np.asarray(lam))
    assert x.shape == (B, N), x.shape
    nc = _get_compiled(lam_f)
    _ensure_concourse()
    from concourse.bass_utils import run_bass_kernel_spmd

    mask = _build_mask_rep()
    in_maps = [
        {"x": x[c * BC:(c + 1) * BC], "mask": mask}
        for c in range(NCORES)
    ]
    res = run_bass_kernel_spmd(nc, in_maps, core_ids=list(range(NCORES)))
    out = np.concatenate([np.asarray(r["out"]) for r in res.results], axis=0)
    return out.astype(np.float32)
